# revision 3
# baseline (speedup 1.0000x reference)
"""GraphSAGE (2-layer, mean aggregation) on 8 Trainium2 NeuronCores.

Strategy (v2):
  - Nodes sharded contiguously across 8 cores by destination row; a per-core
    permutation balances per-128-row-block edge loads across 4 gather
    streams (layer1 lo/hi by src core, layer2 g1/g2 by src half).
  - Aggregation: dma_gather of source rows per edge chunk (128 edges) +
    TensorEngine matmul with an on-chip one-hot (Vector is_equal), PSUM
    accumulated per dst block, scaled by 1/deg at drain.
  - Per-block h-row (layer1) and output (layer2) matmuls are interleaved
    into the aggregation loops so there is no serial tail.
  - The inter-layer AllGather is split in two (rows of each core's first
    25 blocks, then the rest) with Shared outputs; layer-2 edges are
    partitioned by source half (g1/g2) and aggregated in two passes with
    an SBUF accumulator, so g1 gathers overlap the second collective.
"""

import math
from contextlib import ExitStack

import numpy as np
import ml_dtypes

import concourse.bass as bass
import concourse.bacc as bacc
import concourse.mybir as mybir
import concourse.tile as tile
from concourse import bass_utils

P = 128
N_NODES = 50000
N_EDGES = 800000
D_IN = 128
D_HID = 128
D_OUT = 40
N_CORES = 8
ROWS_PER = N_NODES // N_CORES          # 6250
NBLK = math.ceil(ROWS_PER / P)         # 49
NBLK1 = 25                             # blocks in half 1
H1_ROWS = NBLK1 * P                    # 3200
H2_ROWS = ROWS_PER - H1_ROWS           # 3050
NBLK2 = NBLK - NBLK1                   # 24
LO_SPLIT = 31250                       # src core <= 4  -> lo stream (layer 1)
GRP = 32                               # chunks per dma_gather call
GBUFS = 7                              # gather tiles in flight (shared pool)
OBUFS = 4                              # one-hot tiles in flight
NQ = 4                                 # swdge queues

BF16 = ml_dtypes.bfloat16
STREAMS = ("lo", "hi", "g1", "g2")


def _wrap_idxs(idx_flat):
    """dma_gather index layout: idx i lives at [i % 16, i // 16] of a
    16-partition tile, replicated to 128 partitions."""
    n = idx_flat.shape[0]
    assert n % 16 == 0
    w = idx_flat.reshape(n // 16, 16).T.astype(np.int16)  # [16, n/16]
    return np.tile(w, (8, 1))                             # [128, n/16]


def _greedy_assign(items, loads, nbins, caps, targets):
    """Greedy multi-stream balance: items (ids) with per-stream loads
    [n, S]; assign to nbins bins with caps, balancing each stream against
    targets [nbins, S]. Returns list of id-lists."""
    order = np.argsort(-loads.sum(axis=1), kind="stable")
    binloads = np.zeros((nbins, loads.shape[1]))
    cnt = np.zeros(nbins, np.int64)
    bins = [[] for _ in range(nbins)]
    tgt = np.maximum(targets, 1e-9)
    for i in order:
        cost = ((binloads + loads[i][None, :]) / tgt).max(axis=1)
        cost[cnt >= caps] = np.inf
        b = int(np.argmin(cost))
        bins[b].append(items[i])
        binloads[b] += loads[i]
        cnt[b] += 1
    return bins


def preprocess(edge_index):
    """Partition + permute nodes; build per-core per-stream gather chunk
    tables. Chunk counts per (block, stream) are uniform across cores and
    blocks (max over all), required for the shared SPMD program."""
    src0 = np.asarray(edge_index[0], dtype=np.int64)
    dst0 = np.asarray(edge_index[1], dtype=np.int64)

    deg_in = np.bincount(dst0, minlength=N_NODES)
    deg_out = np.bincount(src0, minlength=N_NODES)
    src_core = src0 // ROWS_PER
    lo_edge = src_core <= 4
    in_lo = np.bincount(dst0[lo_edge], minlength=N_NODES)
    in_hi = deg_in - in_lo

    # ---- step 1: split each core's nodes into half1 (3200) / half2 ----
    half1 = np.zeros(N_NODES, bool)
    for k in range(N_CORES):
        nodes = np.arange(k * ROWS_PER, (k + 1) * ROWS_PER)
        loads = np.stack(
            [in_lo[nodes], in_hi[nodes], deg_out[nodes]], axis=1
        ).astype(np.float64)
        tot = loads.sum(axis=0)
        caps = np.array([H1_ROWS, H2_ROWS])
        targets = np.stack([tot * H1_ROWS / ROWS_PER, tot * H2_ROWS / ROWS_PER])
        bins = _greedy_assign(nodes, loads, 2, caps, targets)
        half1[np.asarray(bins[0], np.int64)] = True

    g1_edge = half1[src0]
    in_g1 = np.bincount(dst0[g1_edge], minlength=N_NODES)
    in_g2 = deg_in - in_g1

    # ---- step 2: assign nodes to blocks within each half ----
    perm = np.empty(N_NODES, np.int64)
    for k in range(N_CORES):
        base = k * ROWS_PER
        nodes = np.arange(base, base + ROWS_PER)
        for half, nb, r0 in ((True, NBLK1, 0), (False, NBLK2, H1_ROWS)):
            hn = nodes[half1[nodes] == half]
            loads = np.stack(
                [in_lo[hn], in_hi[hn], in_g1[hn], in_g2[hn]], axis=1
            ).astype(np.float64)
            caps = np.full(nb, P, np.int64)
            caps[-1] = hn.shape[0] - (nb - 1) * P
            tot = loads.sum(axis=0)
            targets = caps[:, None] * (tot[None, :] / hn.shape[0])
            bins = _greedy_assign(hn, loads, nb, caps, targets)
            off = base + r0
            for b in range(nb):
                ids = np.asarray(bins[b], np.int64)
                perm[off : off + ids.shape[0]] = ids
                off += ids.shape[0]

    slot_of = np.empty(N_NODES, np.int64)
    slot_of[perm] = np.arange(N_NODES)
    src = slot_of[src0]
    dst = slot_of[dst0]
    counts = np.bincount(dst, minlength=N_NODES)
    inv_deg = (1.0 / np.maximum(counts, 1)).astype(np.float32)

    # src slot -> gather-table row per stream
    s_core = src // ROWS_PER
    s_loc = src % ROWS_PER
    is_lo = src < LO_SPLIT
    is_g1 = s_loc < H1_ROWS
    g1_row = s_core * H1_ROWS + s_loc
    g2_row = s_core * H2_ROWS + (s_loc - H1_ROWS)

    order = np.argsort(dst, kind="stable")
    s_s, d_s = src[order], dst[order]
    lo_s, g1_s = is_lo[order], is_g1[order]
    g1r_s, g2r_s = g1_row[order], g2_row[order]

    # per (core, block, stream) edge segments
    segs = {}
    nmax = {s: 0 for s in STREAMS}
    for k in range(N_CORES):
        base = k * ROWS_PER
        for b in range(NBLK):
            r0 = base + b * P
            r1 = min(base + ROWS_PER, r0 + P)
            e0 = np.searchsorted(d_s, r0, side="left")
            e1 = np.searchsorted(d_s, r1, side="left")
            sl = slice(e0, e1)
            lo_m, g1_m = lo_s[sl], g1_s[sl]
            segs[(k, b)] = {
                "lo": (s_s[sl][lo_m], d_s[sl][lo_m] - r0),
                "hi": (s_s[sl][~lo_m] - LO_SPLIT, d_s[sl][~lo_m] - r0),
                "g1": (g1r_s[sl][g1_m], d_s[sl][g1_m] - r0),
                "g2": (g2r_s[sl][~g1_m], d_s[sl][~g1_m] - r0),
            }
            for s in STREAMS:
                nmax[s] = max(nmax[s], segs[(k, b)][s][0].shape[0])

    L = {s: max(1, math.ceil(nmax[s] / P)) for s in STREAMS}
    C = {s: NBLK * L[s] for s in STREAMS}

    per_core = []
    for k in range(N_CORES):
        pc = {}
        for s in STREAMS:
            idx = np.zeros((C[s], P), np.int16)
            dstv = np.full((C[s], P), -1.0, np.float32)
            for b in range(NBLK):
                ss, dd = segs[(k, b)][s]
                n = ss.shape[0]
                c0 = b * L[s]
                fl_i = idx[c0 : c0 + L[s]].reshape(-1)
                fl_d = dstv[c0 : c0 + L[s]].reshape(-1)
                fl_i[:n] = ss.astype(np.int16)
                fl_d[:n] = dd.astype(np.float32)
            pc["idx_" + s] = _wrap_idxs(idx.reshape(-1))
            pc["dstv_" + s] = np.ascontiguousarray(dstv.T).astype(BF16)
        pc["invdeg"] = np.tile(
            inv_deg[k * ROWS_PER : (k + 1) * ROWS_PER][None, :], (P, 1)
        ).astype(BF16)
        per_core.append(pc)

    meta = dict(perm=perm, L=L, C=C)
    return meta, per_core


def build_graph(nc, m):
    dt = mybir.dt
    alu = mybir.AluOpType
    act = mybir.ActivationFunctionType
    L, C = m["L"], m["C"]

    x_lo = nc.dram_tensor("x_lo", [LO_SPLIT, D_IN], dt.bfloat16, kind="ExternalInput")
    x_hi = nc.dram_tensor("x_hi", [N_NODES - LO_SPLIT, D_IN], dt.bfloat16,
                          kind="ExternalInput")
    xT_d = nc.dram_tensor("xT", [P, ROWS_PER], dt.bfloat16, kind="ExternalInput")
    idx_d, dstv_d = {}, {}
    for s in STREAMS:
        idx_d[s] = nc.dram_tensor(f"idx_{s}", [P, C[s] * 8], dt.int16,
                                  kind="ExternalInput")
        dstv_d[s] = nc.dram_tensor(f"dstv_{s}", [P, C[s]], dt.bfloat16,
                                   kind="ExternalInput")
    invdeg_d = nc.dram_tensor("invdeg", [P, ROWS_PER], dt.bfloat16,
                              kind="ExternalInput")
    iota_d = nc.dram_tensor("iota", [P, P], dt.bfloat16, kind="ExternalInput")
    w1l_d = nc.dram_tensor("w1lT", [P, D_HID], dt.bfloat16, kind="ExternalInput")
    w1r_d = nc.dram_tensor("w1rT", [P, D_HID], dt.bfloat16, kind="ExternalInput")
    w2l_d = nc.dram_tensor("w2lT", [P, D_OUT], dt.bfloat16, kind="ExternalInput")
    w2r_d = nc.dram_tensor("w2rT", [P, D_OUT], dt.bfloat16, kind="ExternalInput")
    b1_d = nc.dram_tensor("b1r", [1, D_HID], dt.bfloat16, kind="ExternalInput")
    b2_d = nc.dram_tensor("b2r", [1, D_OUT], dt.bfloat16, kind="ExternalInput")
    out_d = nc.dram_tensor("out", [ROWS_PER, D_OUT], dt.float32,
                           kind="ExternalOutput")

    with tile.TileContext(nc) as tc, ExitStack() as ctx:
        sb = ctx.enter_context(tc.tile_pool(name="sb", bufs=1))
        dram = ctx.enter_context(tc.tile_pool(name="dram", bufs=1, space="DRAM"))
        psA = ctx.enter_context(tc.tile_pool(name="psA", bufs=1, space="PSUM"))
        psB = ctx.enter_context(tc.tile_pool(name="psB", bufs=1, space="PSUM"))
        g_p = ctx.enter_context(tc.tile_pool(name="gp", bufs=GBUFS))
        o_p = ctx.enter_context(tc.tile_pool(name="oh", bufs=OBUFS))
        st_p = ctx.enter_context(tc.tile_pool(name="st", bufs=3))

        def load(shape, dtype, src, name):
            t = sb.tile(shape, dtype, name=name)
            nc.sync.dma_start(t[:], src[:])
            return t

        xT_sb = load([P, ROWS_PER], dt.bfloat16, xT_d.ap(), "xT_sb")
        idx_sb, dstv_sb = {}, {}
        for s in STREAMS:
            idx_sb[s] = load([P, C[s] * 8], dt.int16, idx_d[s].ap(), f"idx_{s}_sb")
            dstv_sb[s] = load([P, C[s]], dt.bfloat16, dstv_d[s].ap(),
                              f"dstv_{s}_sb")
        invdeg_sb = load([P, ROWS_PER], dt.bfloat16, invdeg_d.ap(), "invdeg_sb")
        iota_sb = load([P, P], dt.bfloat16, iota_d.ap(), "iota_sb")
        w1l_sb = load([P, D_HID], dt.bfloat16, w1l_d.ap(), "w1l_sb")
        w1r_sb = load([P, D_HID], dt.bfloat16, w1r_d.ap(), "w1r_sb")
        w2l_sb = load([P, D_OUT], dt.bfloat16, w2l_d.ap(), "w2l_sb")
        w2r_sb = load([P, D_OUT], dt.bfloat16, w2r_d.ap(), "w2r_sb")
        b1_sb = load([1, D_HID], dt.bfloat16, b1_d.ap(), "b1_sb")
        b2_sb = load([1, D_OUT], dt.bfloat16, b2_d.ap(), "b2_sb")

        ones_sb = sb.tile([1, 512], dt.bfloat16, name="ones_sb")
        nc.vector.memset(ones_sb[:], 1.0)

        meanT = sb.tile([P, ROWS_PER], dt.bfloat16, name="meanT")
        hT = sb.tile([P, ROWS_PER], dt.bfloat16, name="hT")
        accT = sb.tile([P, ROWS_PER], dt.bfloat16, name="accT")

        hsh = dram.tile([ROWS_PER, D_IN], dt.bfloat16, name="hsh")
        hfullA = dram.tile([N_CORES * H1_ROWS, D_IN], dt.bfloat16, name="hfullA",
                           addr_space="Shared")
        hfullB = dram.tile([N_CORES * H2_ROWS, D_IN], dt.bfloat16, name="hfullB",
                           addr_space="Shared")

        qctr = [0]
        src_ap = {
            "lo": x_lo.ap(),
            "hi": x_hi.ap(),
            "g1": hfullA[:],
            "g2": hfullB[:],
        }
        tiles = {}

        def ensure_group(s, g):
            if (s, g) in tiles:
                return tiles[(s, g)]
            c0, c1 = g * GRP, min(C[s], (g + 1) * GRP)
            nch = c1 - c0
            n = nch * P
            t = g_p.tile([P, GRP, P], dt.bfloat16, tag="gt", name=f"g_{s}")
            nc.gpsimd.dma_gather(
                t[:, :nch, :], src_ap[s],
                idx_sb[s][:, c0 * 8 : c1 * 8],
                n, n, D_IN, elem_step=D_IN, single_packet=False,
                queue_num=qctr[0] % NQ,
            )
            qctr[0] += 1
            ot = o_p.tile([P, GRP, P], dt.bfloat16, tag="ohv", name="ohv")
            for h0 in range(0, nch, GRP // 2):
                h1 = min(nch, h0 + GRP // 2)
                nc.vector.tensor_tensor(
                    ot[:, h0:h1, :],
                    iota_sb[:, None, :].broadcast_to([P, h1 - h0, P]),
                    dstv_sb[s][:, c0 + h0 : c0 + h1, None].broadcast_to(
                        [P, h1 - h0, P]),
                    alu.is_equal,
                )
            tiles[(s, g)] = (t, ot)
            return tiles[(s, g)]

        def accum_block(b, streams, psum):
            ops = []
            for s in streams:
                ops += [(s, c) for c in range(b * L[s], (b + 1) * L[s])]
            for i, (s, c) in enumerate(ops):
                gt, ot = ensure_group(s, c // GRP)
                nc.tensor.matmul(
                    psum[:, :P], lhsT=gt[:, c % GRP, :], rhs=ot[:, c % GRP, :],
                    start=(i == 0), stop=(i == len(ops) - 1),
                )

        # ================= layer 1 =================
        for b in range(NBLK):
            c0 = b * P
            bs = min(P, ROWS_PER - c0)
            ps = psA.tile([P, P], dt.float32, tag="agg", name="ps_agg", bufs=4)
            accum_block(b, ("lo", "hi"), ps)
            nc.vector.tensor_tensor(
                meanT[:, c0 : c0 + bs], ps[:, :bs],
                invdeg_sb[:, c0 : c0 + bs], alu.mult,
            )
            # h rows for this block (row-major, for the collective)
            ps2 = psB.tile([P, 512], dt.float32, tag="ps", name="ps_r", bufs=3)
            nc.tensor.matmul(ps2[:bs, :D_HID], lhsT=meanT[:, c0 : c0 + bs],
                             rhs=w1l_sb[:], start=True, stop=False)
            nc.tensor.matmul(ps2[:bs, :D_HID], lhsT=xT_sb[:, c0 : c0 + bs],
                             rhs=w1r_sb[:], start=False, stop=False)
            nc.tensor.matmul(ps2[:bs, :D_HID], lhsT=ones_sb[:, :bs],
                             rhs=b1_sb[:], start=False, stop=True)
            hrow = st_p.tile([P, D_HID], dt.bfloat16, tag="st", name="hrow")
            nc.scalar.activation(hrow[:bs, :], ps2[:bs, :D_HID], act.Relu)
            nc.sync.dma_start(hsh[c0 : c0 + bs, :], hrow[:bs, :])

        # split AllGather: half 1 (blocks 0..24), then half 2
        nc.gpsimd.collective_compute(
            "AllGather", alu.bypass,
            replica_groups=[list(range(N_CORES))],
            ins=[hsh[0:H1_ROWS, :].opt()], outs=[hfullA[:].opt()],
        )
        nc.gpsimd.collective_compute(
            "AllGather", alu.bypass,
            replica_groups=[list(range(N_CORES))],
            ins=[hsh[H1_ROWS:ROWS_PER, :].opt()], outs=[hfullB[:].opt()],
        )

        # col-major h panels (dense path for layer 2) — overlaps collectives
        for c0 in range(0, ROWS_PER, 512):
            w = min(512, ROWS_PER - c0)
            ps2 = psB.tile([P, 512], dt.float32, tag="ps", name="ps_d", bufs=3)
            nc.tensor.matmul(ps2[:, :w], lhsT=w1l_sb[:], rhs=meanT[:, c0 : c0 + w],
                             start=True, stop=False)
            nc.tensor.matmul(ps2[:, :w], lhsT=w1r_sb[:], rhs=xT_sb[:, c0 : c0 + w],
                             start=False, stop=False)
            nc.tensor.matmul(ps2[:, :w], lhsT=b1_sb[:], rhs=ones_sb[:, :w],
                             start=False, stop=True)
            nc.scalar.activation(hT[:, c0 : c0 + w], ps2[:, :w], act.Relu)

        # ================= layer 2 =================
        # pass 1: g1 chunks -> SBUF accumulator
        for b in range(NBLK):
            c0 = b * P
            bs = min(P, ROWS_PER - c0)
            ps = psA.tile([P, P], dt.float32, tag="agg", name="ps_g1", bufs=4)
            accum_block(b, ("g1",), ps)
            nc.vector.tensor_copy(accT[:, c0 : c0 + bs], ps[:, :bs])

        # pass 2: g2 chunks -> combine, scale, output rows
        for b in range(NBLK):
            c0 = b * P
            bs = min(P, ROWS_PER - c0)
            ps = psA.tile([P, P], dt.float32, tag="agg", name="ps_g2", bufs=4)
            accum_block(b, ("g2",), ps)
            msum = st_p.tile([P, P], dt.bfloat16, tag="ms", name="msum")
            nc.vector.tensor_tensor(msum[:, :bs], ps[:, :bs],
                                    accT[:, c0 : c0 + bs], alu.add)
            meanh = st_p.tile([P, P], dt.bfloat16, tag="mh", name="meanh")
            nc.vector.tensor_tensor(meanh[:, :bs], msum[:, :bs],
                                    invdeg_sb[:, c0 : c0 + bs], alu.mult)
            ps2 = psB.tile([P, 512], dt.float32, tag="ps", name="ps_o", bufs=3)
            nc.tensor.matmul(ps2[:bs, :D_OUT], lhsT=meanh[:, :bs], rhs=w2l_sb[:],
                             start=True, stop=False)
            nc.tensor.matmul(ps2[:bs, :D_OUT], lhsT=hT[:, c0 : c0 + bs],
                             rhs=w2r_sb[:], start=False, stop=False)
            nc.tensor.matmul(ps2[:bs, :D_OUT], lhsT=ones_sb[:, :bs],
                             rhs=b2_sb[:], start=False, stop=True)
            ot = st_p.tile([P, D_OUT], dt.float32, tag="ot", name="ot")
            nc.vector.tensor_copy(ot[:bs, :], ps2[:bs, :D_OUT])
            nc.sync.dma_start(out_d.ap()[c0 : c0 + bs, :], ot[:bs, :])

    return nc


def make_in_maps(inputs, meta, per_core):
    x = np.asarray(inputs["x"], np.float32)[meta["perm"]]
    x_bf = x.astype(BF16)
    w1l = np.asarray(inputs["W1l"], np.float32)
    w1r = np.asarray(inputs["W1r"], np.float32)
    w2l = np.asarray(inputs["W2l"], np.float32)
    w2r = np.asarray(inputs["W2r"], np.float32)
    b1 = np.asarray(inputs["b1"], np.float32)
    b2 = np.asarray(inputs["b2"], np.float32)
    iota = np.tile(np.arange(P, dtype=np.float32)[None, :], (P, 1)).astype(BF16)
    in_maps = []
    for k in range(N_CORES):
        pc = per_core[k]
        im = {
            "x_lo": x_bf[:LO_SPLIT],
            "x_hi": x_bf[LO_SPLIT:],
            "xT": np.ascontiguousarray(
                x[k * ROWS_PER : (k + 1) * ROWS_PER].T).astype(BF16),
            "invdeg": pc["invdeg"],
            "iota": iota,
            "w1lT": np.ascontiguousarray(w1l.T).astype(BF16),
            "w1rT": np.ascontiguousarray(w1r.T).astype(BF16),
            "w2lT": np.ascontiguousarray(w2l.T).astype(BF16),
            "w2rT": np.ascontiguousarray(w2r.T).astype(BF16),
            "b1r": b1[None, :].astype(BF16),
            "b2r": b2[None, :].astype(BF16),
        }
        for s in STREAMS:
            im["idx_" + s] = pc["idx_" + s]
            im["dstv_" + s] = pc["dstv_" + s]
        in_maps.append(im)
    return in_maps


_CACHE = {}


def _compile(meta):
    key = tuple(sorted(meta["L"].items()))
    if key not in _CACHE:
        nc = bacc.Bacc("TRN2", target_bir_lowering=False, debug=False,
                       num_devices=N_CORES, num_swdge_queues=NQ)
        build_graph(nc, meta)
        nc.compile()
        _CACHE[key] = nc
    return _CACHE[key]


def assemble(res, meta):
    out = np.concatenate(
        [np.asarray(res.results[k]["out"]) for k in range(N_CORES)], axis=0
    ).astype(np.float32)
    unperm = np.empty_like(out)
    unperm[meta["perm"]] = out
    return unperm


def kernel(**inputs):
    edge_index = np.asarray(inputs["edge_index"])
    meta, per_core = preprocess(edge_index)
    nc = _compile(meta)
    in_maps = make_in_maps(inputs, meta, per_core)
    res = bass_utils.run_bass_kernel_spmd(
        nc, in_maps, core_ids=list(range(N_CORES))
    )
    return assemble(res, meta)


# revision 8
# speedup vs baseline: 1.0697x; 1.0697x over previous
"""GraphSAGE (2-layer, mean aggregation) on 8 Trainium2 NeuronCores.

Strategy (v3, fp8 pair-gather):
  - Nodes sharded contiguously across 8 cores by destination row; a per-core
    permutation balances per-128-row-block in-edge loads toward 2048/block.
  - Gathered features are stored as fp8(e4m3) PAIRS: table row p holds node
    rows 2p and 2p+1 (256 B). Each edge gathers pair src//2 -> half the DMA
    packets of a bf16 row gather, and pair indices fit int16 (< 25000), so
    no lo/hi index split is needed.
  - Aggregation per 128-edge chunk: two accumulating matmuls into PSUM
    (lhsT = even half / odd half of the gathered pair tile, fp8; rhs = the
    matching parity-masked one-hot, bf16 built on-chip via is_equal).
    PSUM is [feat, dst] column-major; scaled by 1/deg at drain.
  - Both layers share the SAME edge chunk structure, so one idx table and
    one dstv table serve both layers.
  - Layer-1 h rows are computed per block inside the aggregation loop
    (no tail), written to hsh in fp8, exchanged with one AllGather into the
    fp8 pair table for layer 2. The bf16 dense path (x@W1r etc.) keeps
    full precision.
"""

import math
from contextlib import ExitStack

import numpy as np
import ml_dtypes

import concourse.bass as bass
import concourse.bacc as bacc
import concourse.mybir as mybir
import concourse.tile as tile
from concourse import bass_utils

P = 128
N_NODES = 50000
D_IN = 128
D_HID = 128
D_OUT = 40
N_CORES = 8
ROWS_PER = N_NODES // N_CORES          # 6250
NBLK = math.ceil(ROWS_PER / P)         # 49
N_PAIRS = N_NODES // 2                 # 25000
GRP = 32                               # chunks per dma_gather call
GBUFS = 6                              # gather tiles in flight
OBUFS = 4                              # one-hot tiles in flight
NQ = 4                                 # swdge queues
IDX_LOAD_SPLIT = 4

BF16 = ml_dtypes.bfloat16
FP8 = ml_dtypes.float8_e4m3


def _wrap_idxs(idx_flat):
    """dma_gather index layout: idx i lives at [i % 16, i // 16] of a
    16-partition tile, replicated to 128 partitions."""
    n = idx_flat.shape[0]
    assert n % 16 == 0
    w = idx_flat.reshape(n // 16, 16).T.astype(np.int16)  # [16, n/16]
    return np.tile(w, (8, 1))                             # [128, n/16]


def _balance_perm(dst, n_nodes):
    """Per-core permutation evening per-block in-degree (~2048/block)."""
    deg = np.bincount(dst, minlength=n_nodes).astype(np.float64)
    perm = np.empty(n_nodes, np.int64)
    for k in range(N_CORES):
        base = k * ROWS_PER
        nodes = np.arange(base, base + ROWS_PER)
        order = nodes[np.argsort(-deg[nodes], kind="stable")]
        caps = np.full(NBLK, P, np.int64)
        caps[-1] = ROWS_PER - (NBLK - 1) * P
        tot = deg[nodes].sum()
        tgt = caps * (tot / ROWS_PER)
        load = np.zeros(NBLK)
        cnt = np.zeros(NBLK, np.int64)
        bins = [[] for _ in range(NBLK)]
        for n in order:
            cost = (load + deg[n]) / tgt
            cost[cnt >= caps] = np.inf
            b = int(np.argmin(cost))
            bins[b].append(n)
            load[b] += deg[n]
            cnt[b] += 1
        off = base
        for b in range(NBLK):
            ids = np.asarray(bins[b], np.int64)
            perm[off : off + ids.shape[0]] = ids
            off += ids.shape[0]
    return perm


def preprocess(edge_index):
    src0 = np.asarray(edge_index[0], dtype=np.int64)
    dst0 = np.asarray(edge_index[1], dtype=np.int64)
    perm = _balance_perm(dst0, N_NODES)
    slot_of = np.empty(N_NODES, np.int64)
    slot_of[perm] = np.arange(N_NODES)
    src = slot_of[src0]
    dst = slot_of[dst0]
    counts = np.bincount(dst, minlength=N_NODES)
    inv_deg = (1.0 / np.maximum(counts, 1)).astype(np.float32)

    order = np.argsort(dst, kind="stable")
    s_s, d_s = src[order], dst[order]

    # per (core, block) edge segments; uniform-across-cores chunk counts
    seg = {}
    counts_b = np.zeros(NBLK, np.int64)
    for k in range(N_CORES):
        base = k * ROWS_PER
        for b in range(NBLK):
            r0 = base + b * P
            r1 = min(base + ROWS_PER, r0 + P)
            e0 = np.searchsorted(d_s, r0, side="left")
            e1 = np.searchsorted(d_s, r1, side="left")
            seg[(k, b)] = (s_s[e0:e1], d_s[e0:e1] - r0)
            counts_b[b] = max(counts_b[b], (e1 - e0 + P - 1) // P)
    off_b = np.zeros(NBLK + 1, np.int64)
    off_b[1:] = np.cumsum(counts_b)
    C = int(off_b[-1])

    per_core = []
    for k in range(N_CORES):
        idx = np.zeros((C, P), np.int16)
        dstv = np.full((C, P, 2), -1.0, np.float32)
        for b in range(NBLK):
            ss, dd = seg[(k, b)]
            n = ss.shape[0]
            c0 = int(off_b[b])
            nch = int(counts_b[b])
            fl_i = idx[c0 : c0 + nch].reshape(-1)
            fl_d = dstv[c0 : c0 + nch].reshape(-1, 2)
            fl_i[:n] = (ss // 2).astype(np.int16)
            par = (ss % 2).astype(np.int64)
            fl_d[np.arange(n), par] = dd.astype(np.float32)
        per_core.append(dict(
            idx=_wrap_idxs(idx.reshape(-1)),
            # [128 edge slots, C, 2]
            dstv=np.ascontiguousarray(dstv.transpose(1, 0, 2)).astype(BF16),
            invdeg=np.tile(
                inv_deg[k * ROWS_PER : (k + 1) * ROWS_PER][None, :], (P, 1)
            ).astype(BF16),
        ))

    meta = dict(perm=perm, counts_b=tuple(int(c) for c in counts_b),
                off_b=off_b, C=C)
    return meta, per_core


def build_graph(nc, m):
    dt = mybir.dt
    alu = mybir.AluOpType
    act = mybir.ActivationFunctionType
    C = m["C"]
    off_b = m["off_b"]

    xp_d = nc.dram_tensor("xp", [N_PAIRS, 2 * D_IN], dt.float8e4,
                          kind="ExternalInput")
    xT_d = nc.dram_tensor("xT", [P, ROWS_PER], dt.bfloat16, kind="ExternalInput")
    idx_d = nc.dram_tensor("idx", [P, C * 8], dt.int16, kind="ExternalInput")
    dstv_d = nc.dram_tensor("dstv", [P, C, 2], dt.bfloat16, kind="ExternalInput")
    invdeg_d = nc.dram_tensor("invdeg", [P, ROWS_PER], dt.bfloat16,
                              kind="ExternalInput")
    iota_d = nc.dram_tensor("iota", [P, P], dt.bfloat16, kind="ExternalInput")
    w1l_d = nc.dram_tensor("w1lT", [P, D_HID], dt.bfloat16, kind="ExternalInput")
    w1r_d = nc.dram_tensor("w1rT", [P, D_HID], dt.bfloat16, kind="ExternalInput")
    w2l_d = nc.dram_tensor("w2lT", [P, D_OUT], dt.bfloat16, kind="ExternalInput")
    w2r_d = nc.dram_tensor("w2rT", [P, D_OUT], dt.bfloat16, kind="ExternalInput")
    b1_d = nc.dram_tensor("b1r", [1, D_HID], dt.bfloat16, kind="ExternalInput")
    b2_d = nc.dram_tensor("b2r", [1, D_OUT], dt.bfloat16, kind="ExternalInput")
    out_d = nc.dram_tensor("out", [ROWS_PER, D_OUT], dt.float32,
                           kind="ExternalOutput")

    with tile.TileContext(nc) as tc, ExitStack() as ctx:
        sb = ctx.enter_context(tc.tile_pool(name="sb", bufs=1))
        dram = ctx.enter_context(tc.tile_pool(name="dram", bufs=1, space="DRAM"))
        psA = ctx.enter_context(tc.tile_pool(name="psA", bufs=1, space="PSUM"))
        psB = ctx.enter_context(tc.tile_pool(name="psB", bufs=1, space="PSUM"))
        g_p = ctx.enter_context(tc.tile_pool(name="gp", bufs=GBUFS))
        o_p = ctx.enter_context(tc.tile_pool(name="oh", bufs=OBUFS))
        st_p = ctx.enter_context(tc.tile_pool(name="st", bufs=3))

        def load(shape, dtype, src, name, split=1):
            t = sb.tile(shape, dtype, name=name)
            if split == 1:
                nc.sync.dma_start(t[:], src[:])
            else:
                w = shape[1]
                step = (w + split - 1) // split
                for c0 in range(0, w, step):
                    c1 = min(w, c0 + step)
                    nc.sync.dma_start(t[:, c0:c1], src[:, c0:c1])
            return t

        idx_sb = load([P, C * 8], dt.int16, idx_d.ap(), "idx_sb",
                      split=IDX_LOAD_SPLIT)
        dstv3 = sb.tile([P, C, 2], dt.bfloat16, name="dstv3")
        step = (C + IDX_LOAD_SPLIT - 1) // IDX_LOAD_SPLIT
        for c0 in range(0, C, step):
            c1 = min(C, c0 + step)
            nc.sync.dma_start(dstv3[:, c0:c1, :], dstv_d.ap()[:, c0:c1, :])
        xT_sb = load([P, ROWS_PER], dt.bfloat16, xT_d.ap(), "xT_sb")
        invdeg_sb = load([P, ROWS_PER], dt.bfloat16, invdeg_d.ap(), "invdeg_sb")
        iota_sb = load([P, P], dt.bfloat16, iota_d.ap(), "iota_sb")
        w1l_sb = load([P, D_HID], dt.bfloat16, w1l_d.ap(), "w1l_sb")
        w1r_sb = load([P, D_HID], dt.bfloat16, w1r_d.ap(), "w1r_sb")
        w2l_sb = load([P, D_OUT], dt.bfloat16, w2l_d.ap(), "w2l_sb")
        w2r_sb = load([P, D_OUT], dt.bfloat16, w2r_d.ap(), "w2r_sb")
        b1_sb = load([1, D_HID], dt.bfloat16, b1_d.ap(), "b1_sb")
        b2_sb = load([1, D_OUT], dt.bfloat16, b2_d.ap(), "b2_sb")

        ones_sb = sb.tile([1, 512], dt.bfloat16, name="ones_sb")
        nc.vector.memset(ones_sb[:], 1.0)

        meanT = sb.tile([P, ROWS_PER], dt.bfloat16, name="meanT")
        hT = sb.tile([P, ROWS_PER], dt.bfloat16, name="hT")

        hsh = dram.tile([ROWS_PER, D_IN], dt.float8e4, name="hsh")
        hfull = dram.tile([N_PAIRS, 2 * D_IN], dt.float8e4, name="hfull")

        qctr = [0]
        src_ap = [xp_d.ap(), hfull[:]]
        tiles = {}

        def ensure_group(layer, g):
            if (layer, g) in tiles:
                return tiles[(layer, g)]
            c0, c1 = g * GRP, min(C, (g + 1) * GRP)
            nch = c1 - c0
            n = nch * P
            t = g_p.tile([P, GRP, 2 * D_IN], dt.float8e4, tag="gt", name="gt")
            nc.gpsimd.dma_gather(
                t[:, :nch, :], src_ap[layer],
                idx_sb[:, c0 * 8 : c1 * 8],
                n, n, 2 * D_IN, elem_step=2 * D_IN, single_packet=False,
                queue_num=qctr[0] % NQ,
            )
            qctr[0] += 1
            ot = o_p.tile([P, GRP, 2, P], dt.bfloat16, tag="ohv", name="ohv")
            for h0 in range(0, nch, GRP // 2):
                h1 = min(nch, h0 + GRP // 2)
                nc.vector.tensor_tensor(
                    ot[:, h0:h1, :, :],
                    iota_sb[:, None, None, :].broadcast_to([P, h1 - h0, 2, P]),
                    dstv3[:, c0 + h0 : c0 + h1, :, None].broadcast_to(
                        [P, h1 - h0, 2, P]),
                    alu.is_equal,
                )
            tiles[(layer, g)] = (t, ot)
            return tiles[(layer, g)]

        def accum_block(layer, b, psum):
            cs, ce = int(off_b[b]), int(off_b[b + 1])
            last = 2 * (ce - cs) - 1
            i = 0
            for c in range(cs, ce):
                gt, ot = ensure_group(layer, c // GRP)
                j = c % GRP
                for o in range(2):
                    nc.tensor.matmul(
                        psum[:, :P],
                        lhsT=gt[:, j, o * D_IN : (o + 1) * D_IN],
                        rhs=ot[:, j, o, :],
                        start=(i == 0), stop=(i == last),
                    )
                    i += 1

        # ================= layer 1 =================
        for b in range(NBLK):
            c0 = b * P
            bs = min(P, ROWS_PER - c0)
            ps = psA.tile([P, P], dt.float32, tag="agg", name="ps_agg", bufs=4)
            accum_block(0, b, ps)
            nc.vector.tensor_tensor(
                meanT[:, c0 : c0 + bs], ps[:, :bs],
                invdeg_sb[:, c0 : c0 + bs], alu.mult,
            )
            ps2 = psB.tile([P, 512], dt.float32, tag="ps", name="ps_r", bufs=3)
            nc.tensor.matmul(ps2[:bs, :D_HID], lhsT=meanT[:, c0 : c0 + bs],
                             rhs=w1l_sb[:], start=True, stop=False)
            nc.tensor.matmul(ps2[:bs, :D_HID], lhsT=xT_sb[:, c0 : c0 + bs],
                             rhs=w1r_sb[:], start=False, stop=False)
            nc.tensor.matmul(ps2[:bs, :D_HID], lhsT=ones_sb[:, :bs],
                             rhs=b1_sb[:], start=False, stop=True)
            hrow = st_p.tile([P, D_HID], dt.float8e4, tag="st", name="hrow")
            nc.scalar.activation(hrow[:bs, :], ps2[:bs, :D_HID], act.Relu)
            nc.sync.dma_start(hsh[c0 : c0 + bs, :], hrow[:bs, :])

        # col-major bf16 h panels (dense path of layer 2); overlaps AllGather
        for c0 in range(0, ROWS_PER, 512):
            w = min(512, ROWS_PER - c0)
            ps2 = psB.tile([P, 512], dt.float32, tag="ps", name="ps_d", bufs=3)
            nc.tensor.matmul(ps2[:, :w], lhsT=w1l_sb[:], rhs=meanT[:, c0 : c0 + w],
                             start=True, stop=False)
            nc.tensor.matmul(ps2[:, :w], lhsT=w1r_sb[:], rhs=xT_sb[:, c0 : c0 + w],
                             start=False, stop=False)
            nc.tensor.matmul(ps2[:, :w], lhsT=b1_sb[:], rhs=ones_sb[:, :w],
                             start=False, stop=True)
            nc.scalar.activation(hT[:, c0 : c0 + w], ps2[:, :w], act.Relu)

        nc.gpsimd.collective_compute(
            "AllGather", alu.bypass,
            replica_groups=[list(range(N_CORES))],
            ins=[hsh[:].opt()], outs=[hfull[:].opt()],
        )

        # ================= layer 2 =================
        for b in range(NBLK):
            c0 = b * P
            bs = min(P, ROWS_PER - c0)
            ps = psA.tile([P, P], dt.float32, tag="agg", name="ps_agg2", bufs=4)
            accum_block(1, b, ps)
            meanh = st_p.tile([P, P], dt.bfloat16, tag="mh", name="meanh")
            nc.vector.tensor_tensor(meanh[:, :bs], ps[:, :bs],
                                    invdeg_sb[:, c0 : c0 + bs], alu.mult)
            ps2 = psB.tile([P, 512], dt.float32, tag="ps", name="ps_o", bufs=3)
            nc.tensor.matmul(ps2[:bs, :D_OUT], lhsT=meanh[:, :bs], rhs=w2l_sb[:],
                             start=True, stop=False)
            nc.tensor.matmul(ps2[:bs, :D_OUT], lhsT=hT[:, c0 : c0 + bs],
                             rhs=w2r_sb[:], start=False, stop=False)
            nc.tensor.matmul(ps2[:bs, :D_OUT], lhsT=ones_sb[:, :bs],
                             rhs=b2_sb[:], start=False, stop=True)
            ot = st_p.tile([P, D_OUT], dt.float32, tag="ot", name="ot")
            nc.vector.tensor_copy(ot[:bs, :], ps2[:bs, :D_OUT])
            nc.sync.dma_start(out_d.ap()[c0 : c0 + bs, :], ot[:bs, :])

    return nc


def make_in_maps(inputs, meta, per_core):
    x = np.asarray(inputs["x"], np.float32)[meta["perm"]]
    xp = x.astype(FP8).reshape(N_PAIRS, 2 * D_IN)
    w1l = np.asarray(inputs["W1l"], np.float32)
    w1r = np.asarray(inputs["W1r"], np.float32)
    w2l = np.asarray(inputs["W2l"], np.float32)
    w2r = np.asarray(inputs["W2r"], np.float32)
    b1 = np.asarray(inputs["b1"], np.float32)
    b2 = np.asarray(inputs["b2"], np.float32)
    iota = np.tile(np.arange(P, dtype=np.float32)[None, :], (P, 1)).astype(BF16)
    in_maps = []
    for k in range(N_CORES):
        pc = per_core[k]
        in_maps.append({
            "xp": xp,
            "xT": np.ascontiguousarray(
                x[k * ROWS_PER : (k + 1) * ROWS_PER].T).astype(BF16),
            "idx": pc["idx"],
            "dstv": pc["dstv"],
            "invdeg": pc["invdeg"],
            "iota": iota,
            "w1lT": np.ascontiguousarray(w1l.T).astype(BF16),
            "w1rT": np.ascontiguousarray(w1r.T).astype(BF16),
            "w2lT": np.ascontiguousarray(w2l.T).astype(BF16),
            "w2rT": np.ascontiguousarray(w2r.T).astype(BF16),
            "b1r": b1[None, :].astype(BF16),
            "b2r": b2[None, :].astype(BF16),
        })
    return in_maps


_CACHE = {}


def _compile(meta):
    key = meta["counts_b"]
    if key not in _CACHE:
        nc = bacc.Bacc("TRN2", target_bir_lowering=False, debug=False,
                       num_devices=N_CORES, num_swdge_queues=NQ)
        build_graph(nc, meta)
        nc.compile()
        _CACHE[key] = nc
    return _CACHE[key]


def assemble(res, meta):
    out = np.concatenate(
        [np.asarray(res.results[k]["out"]) for k in range(N_CORES)], axis=0
    ).astype(np.float32)
    unperm = np.empty_like(out)
    unperm[meta["perm"]] = out
    return unperm


def kernel(**inputs):
    edge_index = np.asarray(inputs["edge_index"])
    meta, per_core = preprocess(edge_index)
    nc = _compile(meta)
    in_maps = make_in_maps(inputs, meta, per_core)
    res = bass_utils.run_bass_kernel_spmd(
        nc, in_maps, core_ids=list(range(N_CORES))
    )
    return assemble(res, meta)


# revision 13
# speedup vs baseline: 1.1401x; 1.0658x over previous
"""GraphSAGE (2-layer, mean aggregation) on 8 Trainium2 NeuronCores.

Strategy (v3, fp8 pair-gather):
  - Nodes sharded contiguously across 8 cores by destination row; a per-core
    permutation balances per-128-row-block in-edge loads toward 2048/block.
  - Gathered features are stored as fp8(e4m3) PAIRS: table row p holds node
    rows 2p and 2p+1 (256 B). Each edge gathers pair src//2 -> half the DMA
    packets of a bf16 row gather, and pair indices fit int16 (< 25000), so
    no lo/hi index split is needed.
  - Aggregation per 128-edge chunk: two accumulating matmuls into PSUM
    (lhsT = even half / odd half of the gathered pair tile, fp8; rhs = the
    matching parity-masked one-hot, bf16 built on-chip via is_equal).
    PSUM is [feat, dst] column-major; scaled by 1/deg at drain.
  - Both layers share the SAME edge chunk structure, so one idx table and
    one dstv table serve both layers.
  - Layer-1 h rows are computed per block inside the aggregation loop
    (no tail), written to hsh in fp8, exchanged with one AllGather into the
    fp8 pair table for layer 2. The bf16 dense path (x@W1r etc.) keeps
    full precision.
"""

import math
from contextlib import ExitStack

import numpy as np
import ml_dtypes

import concourse.bass as bass
import concourse.bacc as bacc
import concourse.mybir as mybir
import concourse.tile as tile
from concourse import bass_utils

P = 128
N_NODES = 50000
D_IN = 128
D_HID = 128
D_OUT = 40
N_CORES = 8
ROWS_PER = N_NODES // N_CORES          # 6250
NBLK = math.ceil(ROWS_PER / P)         # 49
N_PAIRS = N_NODES // 2                 # 25000
GRP = 32                               # chunks per dma_gather call
GBUFS = 6                              # gather tiles in flight
OBUFS = 4                              # one-hot tiles in flight
NQ = 4                                 # swdge queues
IDX_LOAD_SPLIT = 4

BF16 = ml_dtypes.bfloat16
FP8 = ml_dtypes.float8_e4m3


def _wrap_idxs(idx_flat):
    """dma_gather index layout: idx i lives at [i % 16, i // 16] of a
    16-partition tile, replicated to 128 partitions."""
    n = idx_flat.shape[0]
    assert n % 16 == 0
    w = idx_flat.reshape(n // 16, 16).T.astype(np.int16)  # [16, n/16]
    return np.tile(w, (8, 1))                             # [128, n/16]


def _balance_perm(dst, n_nodes):
    """Per-core permutation evening per-block in-degree (~2048/block)."""
    deg = np.bincount(dst, minlength=n_nodes).astype(np.float64)
    perm = np.empty(n_nodes, np.int64)
    for k in range(N_CORES):
        base = k * ROWS_PER
        nodes = np.arange(base, base + ROWS_PER)
        order = nodes[np.argsort(-deg[nodes], kind="stable")]
        caps = np.full(NBLK, P, np.int64)
        caps[-1] = ROWS_PER - (NBLK - 1) * P
        tot = deg[nodes].sum()
        tgt = caps * (tot / ROWS_PER)
        load = np.zeros(NBLK)
        cnt = np.zeros(NBLK, np.int64)
        bins = [[] for _ in range(NBLK)]
        for n in order:
            cost = (load + deg[n]) / tgt
            cost[cnt >= caps] = np.inf
            b = int(np.argmin(cost))
            bins[b].append(n)
            load[b] += deg[n]
            cnt[b] += 1
        off = base
        for b in range(NBLK):
            ids = np.asarray(bins[b], np.int64)
            perm[off : off + ids.shape[0]] = ids
            off += ids.shape[0]
    return perm


def preprocess(edge_index):
    src0 = np.asarray(edge_index[0], dtype=np.int64)
    dst0 = np.asarray(edge_index[1], dtype=np.int64)
    perm = _balance_perm(dst0, N_NODES)
    slot_of = np.empty(N_NODES, np.int64)
    slot_of[perm] = np.arange(N_NODES)
    src = slot_of[src0]
    dst = slot_of[dst0]
    counts = np.bincount(dst, minlength=N_NODES)
    inv_deg = (1.0 / np.maximum(counts, 1)).astype(np.float32)

    order = np.argsort(dst, kind="stable")
    s_s, d_s = src[order], dst[order]

    # per (core, block) edge segments, evens (src%2==0) first; uniform
    # across-cores chunk counts + per-chunk parity labels (0=E, 1=O, 2=M)
    seg = {}
    counts_b = np.zeros(NBLK, np.int64)
    ne = np.zeros((N_CORES, NBLK), np.int64)
    for k in range(N_CORES):
        base = k * ROWS_PER
        for b in range(NBLK):
            r0 = base + b * P
            r1 = min(base + ROWS_PER, r0 + P)
            e0 = np.searchsorted(d_s, r0, side="left")
            e1 = np.searchsorted(d_s, r1, side="left")
            ss, dd = s_s[e0:e1], d_s[e0:e1] - r0
            ev = (ss % 2) == 0
            ss = np.concatenate([ss[ev], ss[~ev]])
            dd = np.concatenate([dd[ev], dd[~ev]])
            seg[(k, b)] = (ss, dd)
            ne[k, b] = int(ev.sum())
            counts_b[b] = max(counts_b[b], (e1 - e0 + P - 1) // P)
    off_b = np.zeros(NBLK + 1, np.int64)
    off_b[1:] = np.cumsum(counts_b)
    C = int(off_b[-1])

    labels = np.zeros(C, np.int8)
    for b in range(NBLK):
        tmin = int(ne[:, b].min())
        tmax = int(ne[:, b].max())
        for c in range(int(counts_b[b])):
            s0, s1 = c * P, (c + 1) * P
            if s1 <= tmin:
                labels[off_b[b] + c] = 0
            elif s0 >= tmax:
                labels[off_b[b] + c] = 1
            else:
                labels[off_b[b] + c] = 2

    per_core = []
    for k in range(N_CORES):
        idx = np.zeros((C, P), np.int16)
        dstv = np.full((C, P, 2), -1.0, np.float32)
        for b in range(NBLK):
            ss, dd = seg[(k, b)]
            n = ss.shape[0]
            c0 = int(off_b[b])
            nch = int(counts_b[b])
            fl_i = idx[c0 : c0 + nch].reshape(-1)
            fl_d = dstv[c0 : c0 + nch].reshape(-1, 2)
            fl_i[:n] = (ss // 2).astype(np.int16)
            lab = np.repeat(labels[c0 : c0 + nch], P)[:n]
            # pure chunks use lane 0; mixed chunks use lane = src parity
            lane = np.where(lab == 2, ss % 2, 0)
            fl_d[np.arange(n), lane] = dd.astype(np.float32)
        per_core.append(dict(
            idx=_wrap_idxs(idx.reshape(-1)),
            # [128 edge slots, C, 2]
            dstv=np.ascontiguousarray(dstv.transpose(1, 0, 2)).astype(BF16),
            invdeg=np.tile(
                inv_deg[k * ROWS_PER : (k + 1) * ROWS_PER][None, :], (P, 1)
            ).astype(BF16),
        ))

    meta = dict(perm=perm, counts_b=tuple(int(c) for c in counts_b),
                labels=tuple(int(v) for v in labels), off_b=off_b, C=C)
    return meta, per_core


def build_graph(nc, m):
    dt = mybir.dt
    alu = mybir.AluOpType
    act = mybir.ActivationFunctionType
    C = m["C"]
    off_b = m["off_b"]
    labels = m["labels"]

    xp_d = nc.dram_tensor("xp", [N_PAIRS, 2 * D_IN], dt.float8e4,
                          kind="ExternalInput")
    xT_d = nc.dram_tensor("xT", [P, ROWS_PER], dt.bfloat16, kind="ExternalInput")
    idx_d = nc.dram_tensor("idx", [P, C * 8], dt.int16, kind="ExternalInput")
    dstv_d = nc.dram_tensor("dstv", [P, C, 2], dt.bfloat16, kind="ExternalInput")
    invdeg_d = nc.dram_tensor("invdeg", [P, ROWS_PER], dt.bfloat16,
                              kind="ExternalInput")
    iota_d = nc.dram_tensor("iota", [P, P], dt.bfloat16, kind="ExternalInput")
    w1l_d = nc.dram_tensor("w1lT", [P, D_HID], dt.bfloat16, kind="ExternalInput")
    w1r_d = nc.dram_tensor("w1rT", [P, D_HID], dt.bfloat16, kind="ExternalInput")
    w2l_d = nc.dram_tensor("w2lT", [P, D_OUT], dt.bfloat16, kind="ExternalInput")
    w2r_d = nc.dram_tensor("w2rT", [P, D_OUT], dt.bfloat16, kind="ExternalInput")
    b1_d = nc.dram_tensor("b1r", [1, D_HID], dt.bfloat16, kind="ExternalInput")
    b2_d = nc.dram_tensor("b2r", [1, D_OUT], dt.bfloat16, kind="ExternalInput")
    out_d = nc.dram_tensor("out", [ROWS_PER, D_OUT], dt.float32,
                           kind="ExternalOutput")

    with tile.TileContext(nc) as tc, ExitStack() as ctx:
        sb = ctx.enter_context(tc.tile_pool(name="sb", bufs=1))
        dram = ctx.enter_context(tc.tile_pool(name="dram", bufs=1, space="DRAM"))
        psA = ctx.enter_context(tc.tile_pool(name="psA", bufs=1, space="PSUM"))
        psB = ctx.enter_context(tc.tile_pool(name="psB", bufs=1, space="PSUM"))
        g_p = ctx.enter_context(tc.tile_pool(name="gp", bufs=GBUFS))
        o_p = ctx.enter_context(tc.tile_pool(name="oh", bufs=OBUFS))
        st_p = ctx.enter_context(tc.tile_pool(name="st", bufs=3))

        def load(shape, dtype, src, name, split=1):
            t = sb.tile(shape, dtype, name=name)
            if split == 1:
                nc.sync.dma_start(t[:], src[:])
            else:
                w = shape[1]
                step = (w + split - 1) // split
                for c0 in range(0, w, step):
                    c1 = min(w, c0 + step)
                    nc.sync.dma_start(t[:, c0:c1], src[:, c0:c1])
            return t

        idx_sb = load([P, C * 8], dt.int16, idx_d.ap(), "idx_sb",
                      split=IDX_LOAD_SPLIT)
        dstv3 = sb.tile([P, C, 2], dt.bfloat16, name="dstv3")
        step = (C + IDX_LOAD_SPLIT - 1) // IDX_LOAD_SPLIT
        for c0 in range(0, C, step):
            c1 = min(C, c0 + step)
            nc.sync.dma_start(dstv3[:, c0:c1, :], dstv_d.ap()[:, c0:c1, :])
        xT_sb = load([P, ROWS_PER], dt.bfloat16, xT_d.ap(), "xT_sb")
        invdeg_sb = load([P, ROWS_PER], dt.bfloat16, invdeg_d.ap(), "invdeg_sb")
        iota_sb = load([P, P], dt.bfloat16, iota_d.ap(), "iota_sb")
        w1l_sb = load([P, D_HID], dt.bfloat16, w1l_d.ap(), "w1l_sb")
        w1r_sb = load([P, D_HID], dt.bfloat16, w1r_d.ap(), "w1r_sb")
        w2l_sb = load([P, D_OUT], dt.bfloat16, w2l_d.ap(), "w2l_sb")
        w2r_sb = load([P, D_OUT], dt.bfloat16, w2r_d.ap(), "w2r_sb")
        b1_sb = load([1, D_HID], dt.bfloat16, b1_d.ap(), "b1_sb")
        b2_sb = load([1, D_OUT], dt.bfloat16, b2_d.ap(), "b2_sb")

        ones_sb = sb.tile([1, 512], dt.bfloat16, name="ones_sb")
        nc.vector.memset(ones_sb[:], 1.0)

        meanT = sb.tile([P, ROWS_PER], dt.bfloat16, name="meanT")
        hT = sb.tile([P, ROWS_PER], dt.bfloat16, name="hT")

        hsh = dram.tile([ROWS_PER, D_IN], dt.float8e4, name="hsh")
        hfull = dram.tile([N_PAIRS, 2 * D_IN], dt.float8e4, name="hfull")

        qctr = [0]
        src_ap = [xp_d.ap(), hfull[:]]
        tiles = {}

        def ensure_group(layer, g):
            if (layer, g) in tiles:
                return tiles[(layer, g)]
            c0, c1 = g * GRP, min(C, (g + 1) * GRP)
            nch = c1 - c0
            n = nch * P
            t = g_p.tile([P, GRP, 2 * D_IN], dt.float8e4, tag="gt", name="gt")
            nc.gpsimd.dma_gather(
                t[:, :nch, :], src_ap[layer],
                idx_sb[:, c0 * 8 : c1 * 8],
                n, n, 2 * D_IN, elem_step=2 * D_IN, single_packet=False,
                queue_num=qctr[0] % NQ,
            )
            qctr[0] += 1
            ot = o_p.tile([P, GRP, 2, P], dt.bfloat16, tag="ohv", name="ohv")
            # one is_equal per run of same-lane-count chunks (pure: 1 lane,
            # mixed: 2), so pure chunks only build one one-hot.
            h0 = 0
            while h0 < nch:
                nl = 2 if labels[c0 + h0] == 2 else 1
                h1 = h0 + 1
                while (h1 < nch and h1 - h0 < GRP // 2
                       and (2 if labels[c0 + h1] == 2 else 1) == nl):
                    h1 += 1
                nc.vector.tensor_tensor(
                    ot[:, h0:h1, :nl, :],
                    iota_sb[:, None, None, :].broadcast_to([P, h1 - h0, nl, P]),
                    dstv3[:, c0 + h0 : c0 + h1, :nl, None].broadcast_to(
                        [P, h1 - h0, nl, P]),
                    alu.is_equal,
                )
                h0 = h1
            tiles[(layer, g)] = (t, ot)
            return tiles[(layer, g)]

        def accum_block(layer, b, psum):
            cs, ce = int(off_b[b]), int(off_b[b + 1])
            nmm = sum(2 if labels[c] == 2 else 1 for c in range(cs, ce))
            i = 0
            for c in range(cs, ce):
                gt, ot = ensure_group(layer, c // GRP)
                j = c % GRP
                lab = labels[c]
                parities = (0, 1) if lab == 2 else (lab,)
                for o in parities:
                    lane = o if lab == 2 else 0
                    nc.tensor.matmul(
                        psum[:, :P],
                        lhsT=gt[:, j, o * D_IN : (o + 1) * D_IN],
                        rhs=ot[:, j, lane, :],
                        start=(i == 0), stop=(i == nmm - 1),
                    )
                    i += 1

        # ================= layer 1 =================
        for b in range(NBLK):
            c0 = b * P
            bs = min(P, ROWS_PER - c0)
            ps = psA.tile([P, P], dt.float32, tag="agg", name="ps_agg", bufs=4)
            accum_block(0, b, ps)
            nc.vector.tensor_tensor(
                meanT[:, c0 : c0 + bs], ps[:, :bs],
                invdeg_sb[:, c0 : c0 + bs], alu.mult,
            )
            ps2 = psB.tile([P, 512], dt.float32, tag="ps", name="ps_r", bufs=3)
            nc.tensor.matmul(ps2[:bs, :D_HID], lhsT=meanT[:, c0 : c0 + bs],
                             rhs=w1l_sb[:], start=True, stop=False)
            nc.tensor.matmul(ps2[:bs, :D_HID], lhsT=xT_sb[:, c0 : c0 + bs],
                             rhs=w1r_sb[:], start=False, stop=False)
            nc.tensor.matmul(ps2[:bs, :D_HID], lhsT=ones_sb[:, :bs],
                             rhs=b1_sb[:], start=False, stop=True)
            hrow = st_p.tile([P, D_HID], dt.float8e4, tag="st", name="hrow")
            nc.scalar.activation(hrow[:bs, :], ps2[:bs, :D_HID], act.Relu)
            nc.sync.dma_start(hsh[c0 : c0 + bs, :], hrow[:bs, :])

        # col-major bf16 h panels (dense path of layer 2); overlaps AllGather
        for c0 in range(0, ROWS_PER, 512):
            w = min(512, ROWS_PER - c0)
            ps2 = psB.tile([P, 512], dt.float32, tag="ps", name="ps_d", bufs=3)
            nc.tensor.matmul(ps2[:, :w], lhsT=w1l_sb[:], rhs=meanT[:, c0 : c0 + w],
                             start=True, stop=False)
            nc.tensor.matmul(ps2[:, :w], lhsT=w1r_sb[:], rhs=xT_sb[:, c0 : c0 + w],
                             start=False, stop=False)
            nc.tensor.matmul(ps2[:, :w], lhsT=b1_sb[:], rhs=ones_sb[:, :w],
                             start=False, stop=True)
            nc.scalar.activation(hT[:, c0 : c0 + w], ps2[:, :w], act.Relu)

        nc.gpsimd.collective_compute(
            "AllGather", alu.bypass,
            replica_groups=[list(range(N_CORES))],
            ins=[hsh[:].opt()], outs=[hfull[:].opt()],
        )

        # ================= layer 2 =================
        for b in range(NBLK):
            c0 = b * P
            bs = min(P, ROWS_PER - c0)
            ps = psA.tile([P, P], dt.float32, tag="agg", name="ps_agg2", bufs=4)
            accum_block(1, b, ps)
            meanh = st_p.tile([P, P], dt.bfloat16, tag="mh", name="meanh")
            nc.vector.tensor_tensor(meanh[:, :bs], ps[:, :bs],
                                    invdeg_sb[:, c0 : c0 + bs], alu.mult)
            ps2 = psB.tile([P, 512], dt.float32, tag="ps", name="ps_o", bufs=3)
            nc.tensor.matmul(ps2[:bs, :D_OUT], lhsT=meanh[:, :bs], rhs=w2l_sb[:],
                             start=True, stop=False)
            nc.tensor.matmul(ps2[:bs, :D_OUT], lhsT=hT[:, c0 : c0 + bs],
                             rhs=w2r_sb[:], start=False, stop=False)
            nc.tensor.matmul(ps2[:bs, :D_OUT], lhsT=ones_sb[:, :bs],
                             rhs=b2_sb[:], start=False, stop=True)
            ot = st_p.tile([P, D_OUT], dt.float32, tag="ot", name="ot")
            nc.vector.tensor_copy(ot[:bs, :], ps2[:bs, :D_OUT])
            nc.sync.dma_start(out_d.ap()[c0 : c0 + bs, :], ot[:bs, :])

    return nc


def make_in_maps(inputs, meta, per_core):
    x = np.asarray(inputs["x"], np.float32)[meta["perm"]]
    xp = x.astype(FP8).reshape(N_PAIRS, 2 * D_IN)
    w1l = np.asarray(inputs["W1l"], np.float32)
    w1r = np.asarray(inputs["W1r"], np.float32)
    w2l = np.asarray(inputs["W2l"], np.float32)
    w2r = np.asarray(inputs["W2r"], np.float32)
    b1 = np.asarray(inputs["b1"], np.float32)
    b2 = np.asarray(inputs["b2"], np.float32)
    iota = np.tile(np.arange(P, dtype=np.float32)[None, :], (P, 1)).astype(BF16)
    in_maps = []
    for k in range(N_CORES):
        pc = per_core[k]
        in_maps.append({
            "xp": xp,
            "xT": np.ascontiguousarray(
                x[k * ROWS_PER : (k + 1) * ROWS_PER].T).astype(BF16),
            "idx": pc["idx"],
            "dstv": pc["dstv"],
            "invdeg": pc["invdeg"],
            "iota": iota,
            "w1lT": np.ascontiguousarray(w1l.T).astype(BF16),
            "w1rT": np.ascontiguousarray(w1r.T).astype(BF16),
            "w2lT": np.ascontiguousarray(w2l.T).astype(BF16),
            "w2rT": np.ascontiguousarray(w2r.T).astype(BF16),
            "b1r": b1[None, :].astype(BF16),
            "b2r": b2[None, :].astype(BF16),
        })
    return in_maps


_CACHE = {}


def _compile(meta):
    key = (meta["counts_b"], meta["labels"])
    if key not in _CACHE:
        nc = bacc.Bacc("TRN2", target_bir_lowering=False, debug=False,
                       num_devices=N_CORES, num_swdge_queues=NQ)
        build_graph(nc, meta)
        nc.compile()
        _CACHE[key] = nc
    return _CACHE[key]


def assemble(res, meta):
    out = np.concatenate(
        [np.asarray(res.results[k]["out"]) for k in range(N_CORES)], axis=0
    ).astype(np.float32)
    unperm = np.empty_like(out)
    unperm[meta["perm"]] = out
    return unperm


def kernel(**inputs):
    edge_index = np.asarray(inputs["edge_index"])
    meta, per_core = preprocess(edge_index)
    nc = _compile(meta)
    in_maps = make_in_maps(inputs, meta, per_core)
    res = bass_utils.run_bass_kernel_spmd(
        nc, in_maps, core_ids=list(range(N_CORES))
    )
    return assemble(res, meta)
